# revision 47
# baseline (speedup 1.0000x reference)
import numpy as np

VOCAB, EMBED, HIDDEN = 32000, 100, 128
B, T = 1024, 256
NCORES = 8
BS = B // NCORES            # 128 batch rows per core
GCH = 16                    # timesteps per input chunk
NGC = T // GCH              # 16 chunks
GB = 4                      # timesteps per batched inject matmul (one PSUM bank)
NG = T // GB                # 64 inject groups
LAG = 2                     # groups of scan lookahead
HT = 512                    # head tile width (one PSUM bank of f32)
WP = VOCAB // NGC           # wfc columns streamed per scan chunk

_NC_CACHE = {}


def _build_program():
    from contextlib import ExitStack

    import concourse.mybir as mybir
    import concourse.tile as tile
    from concourse.bacc import Bacc

    f32 = mybir.dt.float32
    bf16 = mybir.dt.bfloat16
    Tanh = mybir.ActivationFunctionType.Tanh
    Ident = mybir.ActivationFunctionType.Identity

    nc = Bacc(None, target_bir_lowering=True)
    xg_d = nc.declare_dram_parameter("xg", [128, T * 128], bf16, isOutput=False)
    wih_d = nc.declare_dram_parameter("wihT", [128, 128], bf16, isOutput=False)
    whh_d = nc.declare_dram_parameter("whhT", [128, 128], bf16, isOutput=False)
    wfc_d = nc.declare_dram_parameter("wfcT", [128, VOCAB], bf16, isOutput=False)
    out_d = nc.declare_dram_parameter("out", [BS, VOCAB], bf16, isOutput=True)

    with tile.TileContext(nc) as tc, ExitStack() as ctx:
        singles = ctx.enter_context(tc.tile_pool(name="singles", bufs=1))
        gpool = ctx.enter_context(tc.tile_pool(name="g", bufs=3))
        zpool = ctx.enter_context(tc.tile_pool(name="z", bufs=4, space="PSUM"))
        hpool = ctx.enter_context(tc.tile_pool(name="h", bufs=4))
        opool = ctx.enter_context(tc.tile_pool(name="o", bufs=2, space="PSUM"))
        spool = ctx.enter_context(tc.tile_pool(name="s", bufs=4))

        wih_sb = singles.tile([128, 128], bf16)
        nc.sync.dma_start(wih_sb[:], wih_d[:])
        whh_sb = singles.tile([128, 128], bf16)
        nc.sync.dma_start(whh_sb[:], whh_d[:])
        hlast = singles.tile([128, BS], bf16)
        wfc_sb = singles.tile([128, VOCAB], bf16)

        g_tiles = {}

        def emit_fetch(c):
            g = gpool.tile([128, GCH * 128], bf16)
            nc.sync.dma_start(g[:], xg_d[:, c * GCH * 128 : (c + 1) * GCH * 128])
            nc.sync.dma_start(
                wfc_sb[:, c * WP : (c + 1) * WP], wfc_d[:, c * WP : (c + 1) * WP]
            )
            g_tiles[c] = g

        for c in range(min(3, NGC)):
            emit_fetch(c)

        h_cur = [None]
        zmap = {}

        def front_group(gi):
            g = g_tiles[gi * GB // GCH]
            s0 = (gi * GB % GCH) * 128
            z = zpool.tile([128, GB * 128], f32, space="PSUM")
            if gi == 0:
                nc.tensor.matmul(
                    z[:, 0:128],
                    lhsT=wih_sb[:],
                    rhs=g[:, s0 : s0 + 128],
                    start=True,
                    stop=True,
                )
                nc.tensor.matmul(
                    z[:, 128 : GB * 128],
                    lhsT=wih_sb[:],
                    rhs=g[:, s0 + 128 : s0 + GB * 128],
                    start=True,
                    stop=False,
                )
            else:
                nc.tensor.matmul(
                    z[:],
                    lhsT=wih_sb[:],
                    rhs=g[:, s0 : s0 + GB * 128],
                    start=True,
                    stop=False,
                )
            zmap[gi] = z
            if (gi + 1) * GB % GCH == 0:
                c = (gi + 1) * GB // GCH - 1
                if c + 3 < NGC:
                    emit_fetch(c + 3)

        def back(t):
            z = zmap[t // GB]
            q = (t % GB) * 128
            zq = z[:, q : q + 128]
            if t > 0:
                nc.tensor.matmul(
                    zq, lhsT=whh_sb[:], rhs=h_cur[0][:], start=False, stop=True
                )
            if t < T - 1:
                hn = hpool.tile([128, BS], bf16)
                nc.scalar.activation(hn[:], zq, Tanh)
                h_cur[0] = hn
            else:
                nc.scalar.activation(hlast[:], zq, Tanh)
            if t % GB == GB - 1:
                del zmap[t // GB]

        for gi in range(NG):
            front_group(gi)
            if gi >= LAG:
                for t in range((gi - LAG) * GB, (gi - LAG + 1) * GB):
                    back(t)
        for gi in range(NG - LAG, NG):
            for t in range(gi * GB, (gi + 1) * GB):
                back(t)

        off = 0
        j = 0
        while off < VOCAB:
            w = min(HT, VOCAB - off)
            o = opool.tile([128, w], f32, space="PSUM")
            nc.tensor.matmul(
                o[:], lhsT=hlast[:], rhs=wfc_sb[:, off : off + w], start=True, stop=True
            )
            s = spool.tile([128, w], bf16)
            if j % 2 == 0:
                nc.scalar.activation(s[:], o[:], Ident)
            else:
                nc.vector.tensor_copy(s[:], o[:])
            nc.sync.dma_start(out_d[:, off : off + w], s[:])
            off += w
            j += 1

    return nc


def get_nc():
    if "nc" not in _NC_CACHE:
        nc = _build_program()
        nc.finalize()
        _NC_CACHE["nc"] = nc
    return _NC_CACHE["nc"]


def make_in_maps(x, emb, W_ih, W_hh, b_ih, b_hh, W_fc, b_fc):
    import ml_dtypes

    bf16 = ml_dtypes.bfloat16

    emb_ext = np.zeros((VOCAB, 128), np.float32)
    emb_ext[:, :EMBED] = emb
    emb_ext[:, EMBED] = 1.0
    emb_ext[:, EMBED + 1] = 1.0

    wihT = np.zeros((128, 128), np.float32)
    wihT[:EMBED] = W_ih.T
    wihT[EMBED] = b_ih
    wihT[EMBED + 1] = b_hh
    wihT = wihT.astype(bf16)

    whhT = np.ascontiguousarray(W_hh.T).astype(bf16)
    wfcT = np.ascontiguousarray(W_fc.T).astype(bf16)

    in_maps = []
    for core in range(NCORES):
        xs = np.asarray(x[core * BS : (core + 1) * BS])  # [128, 256]
        xg = emb_ext[xs]                                 # [128, 256, 128] (b, t, f)
        xg = np.ascontiguousarray(np.transpose(xg, (2, 1, 0))).astype(bf16)
        xg = xg.reshape(128, T * 128)
        in_maps.append(
            {
                "xg": xg,
                "wihT": wihT,
                "whhT": whhT,
                "wfcT": wfcT,
            }
        )
    return in_maps


def kernel(x, emb, W_ih, W_hh, b_ih, b_hh, W_fc, b_fc):
    from concourse.bass_utils import run_bass_kernel_spmd

    nc = get_nc()
    in_maps = make_in_maps(x, emb, W_ih, W_hh, b_ih, b_hh, W_fc, b_fc)
    res = run_bass_kernel_spmd(nc, in_maps, list(range(NCORES)))
    out = np.concatenate(
        [np.asarray(res.results[i]["out"]).astype(np.float32) for i in range(NCORES)],
        axis=0,
    )
    out += np.asarray(b_fc, np.float32)[None, :]
    return out
